# revision 16
# baseline (speedup 1.0000x reference)
"""Trainium2 Bass kernel for nn_BinaryTreeLogicNet.

Computes, for x:[B,256], W_leaf:[256,256], weights:[255,2], biases:[255],
w_out:[1,1], b_out:[1]:

    leaf = sigmoid(x @ W_leaf.T - 2)                       # (B, 256)
    8-level pairwise tree reduce with generalized-gcd nodes # (B, 1)
    out  = sigmoid(root * w_out + b_out)

Key transformations (all host-side constant folding; math exact to ~1e-6):
  - every tree value is positive (sigmoid outputs x positive weights), so
    the |.| is a no-op and min/max are plain min/max;
  - lam*min + (1-lam)*max  =  A*(l+r) + C*max(l,r), A = lam*k, C = k(1-2lam),
    where the consumer weight k of the NEXT level folds in (w_out at root);
  - each level's A further folds into its children's stored scale (sigma
    chain anchored at RHO), so a level is just  node = (l+r) + Chat*max(l,r)
    with Chat = C/A: 4 DVE tensor_tensor ops per level, all fp16 2x mode;
  - levels are stored in bit-reversed node order, which makes every level's
    children the two contiguous halves of the previous buffer, elementwise
    aligned with the outputs: no strided access anywhere;
  - the reference's +EPS contributes k*EPS per node (~1e-6 absolute at the
    root, far below fp16 storage rounding) and is dropped.

Schedule: the kernel is DVE-throughput-bound (the tree is ~43us of
tensor_tensor streaming per core at the fp16 2x mode rate). Per x-group
only the wint multiply + level 0 run on the DVE (5 ops); levels 1..6 and
the root run once over all 64 tiles at the end with large free dims. Small
leading groups get the DVE started ~3us into the kernel.

Sharding: pure data parallel over the batch dim across 8 cores. x is
transposed (and cast to bf16) on the host so the matmul contraction dim
lands on SBUF partitions and all DMA is contiguous.
"""

import numpy as np

import concourse.bass as bass
import concourse.bacc as bacc
import concourse.mybir as mybir
import concourse.tile as tile
from concourse.bass_utils import run_bass_kernel_spmd

# ---- problem geometry (hardcoded per contract) ----
B, L = 65536, 256
N_CORES = 8
BS = B // N_CORES            # 8192 rows per core
TILES = BS // 128            # 64 tiles of 128 rows
# Tree-group sizes (in 128-row tiles). Per group only wint + level 0 run on
# the DVE (5 ops); levels 1..root run once over all tiles at the end (the
# "deep pass"), so groups can be small without paying per-op overhead 8x.
# Small leading groups let the DVE start ~3us in instead of ~18us.
GROUP_SIZES = [1, 1, 2, 4, 8, 16, 32]
XSUB = 8                     # x tiles per DMA chunk (pipelining granularity)
ACT_SUB = 4                  # tiles per PSUM tile / activation (FD=1024)
RHO = 128.0                  # global pow2 rescale for the A-folded tree
CST_W = 1536                 # cst row: wint(256) | Chat_cat(254) | nm Chat
NM_OFFS = {4: 512, 5: 1024, 6: 1280}   # node-major Chat blocks (m x 64)
# wint split point per group size: tiles [gs:T] multiply on gpsimd (runs
# concurrently with the DVE, which keeps tiles [0:gs] plus all level-0 ops)
GP_SPLIT = {}  # gpsimd offload measured net-negative: DVE TTs lose rd1 bandwidth

EPS = 1e-6
SHARPNESS = 1.0
BIAS_SHIFT = -2.0

# dtypes (tunable): matmul path and tree path
MM_DT = mybir.dt.bfloat16    # halves x DMA and runs PE at full rate
TREE_DT = mybir.dt.float16   # 16-bit => DVE 2x mode on contiguous tensor ops
CST_DT = mybir.dt.float16    # tree constants (must match tree dtype for tt)


def _sigmoid(z):
    return 1.0 / (1.0 + np.exp(-z))


def _levels():
    """[(offset, m)] for m = 128, 64, ..., 1 into the weights/biases arrays."""
    out, off, m = [], 0, 128
    while m >= 1:
        out.append((off, m))
        off += m
        m //= 2
    return out


def _bitrev(n):
    """Bit-reversal permutation of 0..n-1 (involution)."""
    bits = n.bit_length() - 1
    out = np.zeros(n, np.int64)
    for j in range(n):
        r, x = 0, j
        for _ in range(bits):
            r = (r << 1) | (x & 1)
            x >>= 1
        out[j] = r
    return out


def prep_consts(weights, biases, w_out):
    """Host-folded per-node constants, A-folded, in bit-reversed order.

    Each node's A = lam*k coefficient is pushed down into its children's
    scales (sigma chain, anchored at sigma_root = RHO for fp16 range), so
    levels 0..6 need only  node = (l + r) + Chat*max(l, r)  with
    Chat = C/A.  The root level keeps explicit A'=A/RHO, C'=C/RHO.

    Level li stores its m output nodes at position q = bitrev(j); with leaves
    stored bit-reversed too, every level's children are the two contiguous
    halves of the previous buffer, elementwise aligned with the outputs.

    Returns (wint[256], Chat_cat[254], a7, c7) in float64 (orig math),
    br-permuted, ready to cast.
    """
    w = weights.astype(np.float64)
    b = biases.astype(np.float64)
    lv = _levels()
    A_lv, C_lv = [], []
    for li, (off, m) in enumerate(lv):
        lam = _sigmoid(b[off : off + m])
        if li + 1 < len(lv):
            noff, nm = lv[li + 1]
            k = np.empty(m, np.float64)
            k[0::2] = w[noff : noff + nm, 0]
            k[1::2] = w[noff : noff + nm, 1]
        else:
            k = np.full(m, float(w_out[0, 0]), np.float64)
        A_lv.append(lam * k)
        C_lv.append(k * (1.0 - 2.0 * lam))
    # sigma chain: sig[li][j] = scale of level-li node j's stored value.
    # Levels 0..6 use the A-folded 4-op form, so each level's A goes into its
    # children's sigma; the root (level 7) keeps its A explicit, so its
    # children carry only the RHO range-rescale.
    sig = [None] * 7
    sig[6] = np.full(2, RHO)
    for li in range(5, -1, -1):
        j = np.arange(128 >> li)
        sig[li] = sig[li + 1][j >> 1] * A_lv[li + 1][j >> 1]
    l_idx = np.arange(256)
    leaf_sig = sig[0][l_idx >> 1] * A_lv[0][l_idx >> 1]

    wint = np.empty(256, np.float64)
    wint[0::2] = w[0:128, 0]
    wint[1::2] = w[0:128, 1]
    wint = (wint * leaf_sig)[_bitrev(256)]

    Chat_parts = [
        (C_lv[li] / A_lv[li])[_bitrev(128 >> li)] for li in range(7)
    ]
    a7 = float(A_lv[7][0] / RHO)
    c7 = float(C_lv[7][0] / RHO)
    return wint, np.concatenate(Chat_parts), a7, c7


def host_emulate(x, W_leaf, weights, biases, w_out, b_out, dtype=np.float32):
    """Pure-numpy emulation of the exact kernel math/layout (for validation)."""
    wint, Chat_cat, a7, c7 = prep_consts(weights, biases, w_out)
    W_perm = W_leaf[_bitrev(256)]  # leaf l lands in column bitrev(l)
    leaf = _sigmoid(
        (x.astype(np.float32) @ W_perm.T.astype(np.float32)) + np.float32(BIAS_SHIFT)
    ).astype(dtype)
    cur = (leaf * wint.astype(dtype)).astype(dtype)
    off = 0
    for li in range(7):
        m = 128 >> li
        l_, r_ = cur[:, 0:m], cur[:, m : 2 * m]
        s = (l_ + r_).astype(dtype)
        mx = np.maximum(l_, r_)
        Ch = Chat_cat[off : off + m].astype(dtype)
        cur = (s + (mx * Ch).astype(dtype)).astype(dtype)
        off += m
    l_, r_ = cur[:, 0:1], cur[:, 1:2]
    s = (l_ + r_).astype(dtype)
    mx = np.maximum(l_, r_)
    cur = ((s * dtype(a7)).astype(dtype) + (mx * dtype(c7)).astype(dtype)).astype(dtype)
    return _sigmoid(cur.astype(np.float32) + np.float32(b_out[0]))


def build_nc(b_out_val, a7, c7):
    """Build the per-core Bass program (SPMD; same NEFF on all cores)."""
    nc = bacc.Bacc("TRN2", target_bir_lowering=False, debug=False)

    xt = nc.dram_tensor("xt", [128, 2, BS], MM_DT, kind="ExternalInput")
    wt = nc.dram_tensor("wt", [128, 2, 256], MM_DT, kind="ExternalInput")
    # cst rows all identical: [wint'(256) | Chat_cat(254) | pad]
    cst = nc.dram_tensor("cst", [128, CST_W], CST_DT, kind="ExternalInput")
    outp = nc.dram_tensor("out", [128, TILES], mybir.dt.float32, kind="ExternalOutput")

    CHAT_OFF = 256
    XCOLS = XSUB * 128

    with tile.TileContext(nc) as tc:
        with (
            tc.tile_pool(name="const", bufs=1) as constp,
            tc.tile_pool(name="xload", bufs=5) as xp,
            tc.tile_pool(name="leaf", bufs=3) as leafp,
            tc.tile_pool(name="work", bufs=1) as workp,
            tc.tile_pool(name="psum", bufs=3, space="PSUM") as psp,
            tc.tile_pool(name="warmpsum", bufs=1, space="PSUM") as wpsp,
        ):
            # x chunk DMAs go first on the sync queue (they gate everything);
            # wt/cst ride the gpsimd queue in parallel. Each DMA has a ~3.4us
            # descriptor-latency floor, so both planes of a chunk share one.
            # DMA completion latency is a fixed ~4us, so the two gating
            # transfers (first x chunk, wt) go out immediately on separate
            # queues; cst follows on gpsimd (only needed by the first wint).
            xab_first = xp.tile([128, 2, XSUB * 128], MM_DT, tag="xab")
            fcols = GROUP_SIZES[0] * 128
            nc.sync.dma_start(
                out=xab_first[:, :, 0:fcols], in_=xt.ap()[:, :, 0:fcols]
            )
            wt_sb = constp.tile([128, 2, 256], MM_DT)
            cst_sb = constp.tile([128, CST_W], CST_DT)
            nc.gpsimd.dma_start(out=wt_sb[:, :, :], in_=wt.ap())
            nc.sync.dma_start(out=cst_sb[:, :], in_=cst.ap())
            roots = constp.tile([128, TILES], TREE_DT)
            bias_shift = constp.tile([128, 1], mybir.dt.float32)
            nc.vector.memset(bias_shift[:, :], float(BIAS_SHIFT))
            bias_out = constp.tile([128, 1], mybir.dt.float32)
            nc.vector.memset(bias_out[:, :], float(b_out_val))
            # Warm the ACT sigmoid table at t=0 so the ~2.7us table load
            # overlaps the first x DMA + matmuls instead of serializing in
            # front of the first real activation.
            warm = constp.tile([128, 1], mybir.dt.float32)
            nc.scalar.activation(
                out=warm[:, :],
                in_=bias_out[:, :],
                func=mybir.ActivationFunctionType.Sigmoid,
                bias=bias_out[:, :],
                scale=1.0,
            )
            # Warm the PE HAM clock gate during the initial DMA wait: ~16
            # back-to-back matmuls on a zeroed SBUF tile give the >3.4us of
            # sustained PE activity that flips the clock from 1.2 to 2.4 GHz
            # before the first real matmul issues.
            dummy = constp.tile([128, 256], MM_DT)
            nc.vector.memset(dummy[:, :], 0.0)
            wps = wpsp.tile([128, 2, 256], mybir.dt.float32, tag="warmps")
            for _ in range(8):
                nc.tensor.matmul(
                    wps[:, 0, :], dummy[:, 0:128], dummy[:, :],
                    start=True, stop=True,
                )
                nc.tensor.matmul(
                    wps[:, 1, :], dummy[:, 0:128], dummy[:, :],
                    start=True, stop=True,
                )
            # L0 outputs for all tiles, bit-reversed node order (the deep
            # pass consumes halves of the node dim, elementwise aligned).
            l0out = constp.tile([128, TILES, 128], TREE_DT)

            def bconst(lo, n, T):
                """cst slice [128, n] broadcast to [128, T, n]."""
                return (
                    cst_sb[:, lo : lo + n]
                    .rearrange("p (o w) -> p o w", o=1)
                    .broadcast_to([128, T, n])
                )

            gstart = 0
            chunk_no = 0
            for T in GROUP_SIZES:
                # x-load chunks of <= XSUB tiles
                chunks, rem = [], T
                while rem > 0:
                    c = min(XSUB, rem)
                    chunks.append(c)
                    rem -= c
                leafg = leafp.tile([128, T, 256], TREE_DT, tag="leafg")
                done = 0
                for xsub in chunks:
                    xcols = xsub * 128
                    xoff = gstart + done * 128
                    if gstart == 0 and done == 0:
                        xab = xab_first
                    else:
                        xab = xp.tile([128, 2, XCOLS], MM_DT, tag="xab")
                        dmaq = nc.sync if (chunk_no % 2 == 0) else nc.gpsimd
                        dmaq.dma_start(
                            out=xab[:, :, 0:xcols],
                            in_=xt.ap()[:, :, xoff : xoff + xcols],
                        )
                        chunk_no += 1
                    asub = min(ACT_SUB, xsub)
                    assert xsub % asub == 0
                    for tp in range(xsub // asub):
                        ps = psp.tile([128, asub, 256], mybir.dt.float32, tag="ps")
                        for half in range(asub):
                            tl = asub * tp + half  # tile within sub-block
                            bsl = slice(tl * 128, (tl + 1) * 128)
                            nc.tensor.matmul(
                                ps[:, half, :],
                                xab[:, 0, bsl],
                                wt_sb[:, 0, :],
                                start=True,
                                stop=False,
                            )
                            nc.tensor.matmul(
                                ps[:, half, :],
                                xab[:, 1, bsl],
                                wt_sb[:, 1, :],
                                start=False,
                                stop=True,
                            )
                        t0 = done + asub * tp
                        nc.scalar.activation(
                            out=leafg[:, t0 : t0 + asub, :],
                            in_=ps[:, :, :],
                            func=mybir.ActivationFunctionType.Sigmoid,
                            bias=bias_shift[:, :],
                            scale=float(SHARPNESS),
                        )
                    done += xsub

                # Per-group DVE work: wint mult + level 0 only (5 ops).
                # pg = leaf * wint' (sigma-folded level-0 child weights),
                # then level-0 nodes go straight into the shared l0out slab.
                pg = workp.tile([128, T, 256], TREE_DT, tag="pg", bufs=2)
                scr = workp.tile([128, T, 384], TREE_DT, tag="scratch", bufs=1)
                gs = GP_SPLIT.get(T, T)   # tiles [gs:T] go to gpsimd
                if gs < T:
                    nc.gpsimd.tensor_tensor(
                        out=pg[:, gs:T, :],
                        in0=leafg[:, gs:T, :],
                        in1=bconst(0, 256, T - gs),
                        op=mybir.AluOpType.mult,
                    )
                nc.vector.tensor_tensor(
                    out=pg[:, 0:gs, :],
                    in0=leafg[:, 0:gs, :],
                    in1=bconst(0, 256, gs),
                    op=mybir.AluOpType.mult,
                )
                le = pg[:, :, 0:128]
                ro = pg[:, :, 128:256]
                s = scr[:, :, 0:128]
                mx = scr[:, :, 128:256]
                q2 = scr[:, :, 256:384]
                nc.vector.tensor_tensor(out=s, in0=le, in1=ro, op=mybir.AluOpType.add)
                nc.vector.tensor_tensor(out=mx, in0=le, in1=ro, op=mybir.AluOpType.max)
                nc.vector.tensor_tensor(
                    out=q2,
                    in0=mx,
                    in1=bconst(CHAT_OFF, 128, T),
                    op=mybir.AluOpType.mult,
                )
                gt = gstart // 128
                nc.vector.tensor_tensor(
                    out=l0out[:, gt : gt + T, :],
                    in0=s,
                    in1=q2,
                    op=mybir.AluOpType.add,
                )
                gstart += T * 128

            # Deep pass: levels 1..6 once over all tiles (large FD => the
            # 58-cycle per-op startup is paid 24x instead of 8x per group),
            # then the root level and the output sigmoid. Levels 1..3 run
            # tile-major; level 3's output is written node-major (a strided
            # 1x write, +0.5us) so levels 4..6 and the root see contiguous
            # step-1 runs of 64 instead of runs of <=8 (saves ~2.5us).
            scrd = workp.tile([128, TILES, 192], TREE_DT, tag="dscratch", bufs=1)
            off = 128
            for li in range(1, 7):
                m = 128 >> li
                le = l0out[:, :, 0:m]
                ro = l0out[:, :, m : 2 * m]
                s = scrd[:, :, 0:m]
                mx = scrd[:, :, 64 : 64 + m]
                q2 = scrd[:, :, 128 : 128 + m]
                nc.vector.tensor_tensor(out=s, in0=le, in1=ro, op=mybir.AluOpType.add)
                nc.vector.tensor_tensor(out=mx, in0=le, in1=ro, op=mybir.AluOpType.max)
                nc.vector.tensor_tensor(
                    out=q2,
                    in0=mx,
                    in1=bconst(CHAT_OFF + off, m, TILES),
                    op=mybir.AluOpType.mult,
                )
                nc.vector.tensor_tensor(
                    out=l0out[:, :, 0:m], in0=s, in1=q2, op=mybir.AluOpType.add
                )
                off += m

            # root level: explicit A' = A/RHO, C' = C/RHO immediates
            s = scrd[:, :, 0:1]
            mx = scrd[:, :, 64:65]
            q2 = scrd[:, :, 128:129]
            nc.vector.tensor_tensor(
                out=s,
                in0=l0out[:, :, 0:1],
                in1=l0out[:, :, 1:2],
                op=mybir.AluOpType.add,
            )
            nc.vector.tensor_tensor(
                out=mx,
                in0=l0out[:, :, 0:1],
                in1=l0out[:, :, 1:2],
                op=mybir.AluOpType.max,
            )
            nc.vector.tensor_scalar_mul(out=q2, in0=mx, scalar1=float(c7))
            rsl = roots[:, :].rearrange("p (t o) -> p t o", o=1)
            nc.vector.scalar_tensor_tensor(
                out=rsl,
                in0=s,
                scalar=float(a7),
                in1=q2,
                op0=mybir.AluOpType.mult,
                op1=mybir.AluOpType.add,
            )

            final = constp.tile([128, TILES], mybir.dt.float32)
            nc.scalar.activation(
                out=final[:, :],
                in_=roots[:, :],
                func=mybir.ActivationFunctionType.Sigmoid,
                bias=bias_out[:, :],
                scale=1.0,
            )
            nc.sync.dma_start(out=outp.ap(), in_=final[:, :])

    nc.compile()
    return nc


def make_in_maps(x, W_leaf, weights, biases, w_out):
    """Host-side sharding + layout prep. Returns per-core input dicts."""
    import ml_dtypes

    np_mm = ml_dtypes.bfloat16
    np_cst = np.float16
    wint, Chat_cat, a7, c7 = prep_consts(weights, biases, w_out)

    cst_row = np.zeros(CST_W, np_cst)
    cst_row[0:256] = wint.astype(np_cst)
    cst_row[256 : 256 + 254] = Chat_cat.astype(np_cst)
    # node-major Chat for deep levels 4..6: value per node, repeated over
    # the 64-tile inner dim so the kernel-side AP is step-1
    lv_off = {1: 128, 2: 192, 3: 224, 4: 240, 5: 248, 6: 252}
    for li, nmo in NM_OFFS.items():
        m = 128 >> li
        ch = Chat_cat[lv_off[li] : lv_off[li] + m]
        cst_row[nmo : nmo + m * TILES] = np.repeat(ch, TILES).astype(np_cst)
    cst = np.ascontiguousarray(np.broadcast_to(cst_row, (128, CST_W)))

    # leaf l lands in column bitrev(l); wt[p, c, l] = W_perm[l, c*128 + p]
    W_perm = W_leaf[_bitrev(256)]
    WT = np.ascontiguousarray(W_perm.T.astype(np_mm))  # [256, 256] (k, l)
    wt_host = np.ascontiguousarray(WT.reshape(2, 128, 256).transpose(1, 0, 2))

    xT = np.ascontiguousarray(x.T.astype(np_mm))  # [256, B]
    in_maps = []
    for c in range(N_CORES):
        sh = np.ascontiguousarray(
            xT[:, c * BS : (c + 1) * BS].reshape(2, 128, BS).transpose(1, 0, 2)
        )
        in_maps.append({"xt": sh, "wt": wt_host, "cst": cst})
    return in_maps, a7, c7


def gather_out(results):
    """Per-core [128, TILES] outputs -> full [B, 1]."""
    full = np.empty((B, 1), np.float32)
    for c in range(N_CORES):
        r = np.asarray(results[c]["out"])  # [128, TILES]
        full[c * BS : (c + 1) * BS, 0] = r.T.reshape(BS)
    return full


def kernel(x, W_leaf, weights, biases, w_out, b_out, _run_kwargs=None):
    x = np.asarray(x, dtype=np.float32)
    W_leaf = np.asarray(W_leaf, dtype=np.float32)
    weights = np.asarray(weights, dtype=np.float32)
    biases = np.asarray(biases, dtype=np.float32)
    w_out = np.asarray(w_out, dtype=np.float32)
    b_out = np.asarray(b_out, dtype=np.float32)
    in_maps, a7, c7 = make_in_maps(x, W_leaf, weights, biases, w_out)
    nc = build_nc(float(b_out[0]), a7, c7)
    kw = dict(_run_kwargs or {})
    res = run_bass_kernel_spmd(nc, in_maps, core_ids=list(range(N_CORES)), **kw)
    out = gather_out(res.results)
    if _run_kwargs is not None:
        kernel.last_results = res
    return out


# revision 17
# speedup vs baseline: 1.0055x; 1.0055x over previous
"""Trainium2 Bass kernel for nn_BinaryTreeLogicNet.

Computes, for x:[B,256], W_leaf:[256,256], weights:[255,2], biases:[255],
w_out:[1,1], b_out:[1]:

    leaf = sigmoid(x @ W_leaf.T - 2)                       # (B, 256)
    8-level pairwise tree reduce with generalized-gcd nodes # (B, 1)
    out  = sigmoid(root * w_out + b_out)

Key transformations (all host-side constant folding; math exact to ~1e-6):
  - every tree value is positive (sigmoid outputs x positive weights), so
    the |.| is a no-op and min/max are plain min/max;
  - lam*min + (1-lam)*max  =  A*(l+r) + C*max(l,r), A = lam*k, C = k(1-2lam),
    where the consumer weight k of the NEXT level folds in (w_out at root);
  - each level's A further folds into its children's stored scale (sigma
    chain anchored at RHO), so a level is just  node = (l+r) + Chat*max(l,r)
    with Chat = C/A: 4 DVE tensor_tensor ops per level, all fp16 2x mode;
  - levels are stored in bit-reversed node order, which makes every level's
    children the two contiguous halves of the previous buffer, elementwise
    aligned with the outputs: no strided access anywhere;
  - the reference's +EPS contributes k*EPS per node (~1e-6 absolute at the
    root, far below fp16 storage rounding) and is dropped.

Schedule: the kernel is DVE-throughput-bound (the tree is ~43us of
tensor_tensor streaming per core at the fp16 2x mode rate). Per x-group
only the wint multiply + level 0 run on the DVE (5 ops); levels 1..6 and
the root run once over all 64 tiles at the end with large free dims. Small
leading groups get the DVE started ~3us into the kernel.

Sharding: pure data parallel over the batch dim across 8 cores. x is
transposed (and cast to bf16) on the host so the matmul contraction dim
lands on SBUF partitions and all DMA is contiguous.
"""

import numpy as np

import concourse.bass as bass
import concourse.bacc as bacc
import concourse.mybir as mybir
import concourse.tile as tile
from concourse.bass_utils import run_bass_kernel_spmd

# ---- problem geometry (hardcoded per contract) ----
B, L = 65536, 256
N_CORES = 8
BS = B // N_CORES            # 8192 rows per core
TILES = BS // 128            # 64 tiles of 128 rows
# Tree-group sizes (in 128-row tiles). Per group only wint + level 0 run on
# the DVE (5 ops); levels 1..root run once over all tiles at the end (the
# "deep pass"), so groups can be small without paying per-op overhead 8x.
# Small leading groups let the DVE start ~3us in instead of ~18us.
GROUP_SIZES = [2, 2, 4, 8, 16, 32]
XSUB = 8                     # x tiles per DMA chunk (pipelining granularity)
ACT_SUB = 4                  # tiles per PSUM tile / activation (FD=1024)
RHO = 128.0                  # global pow2 rescale for the A-folded tree
CST_W = 1536                 # cst row: wint(256) | Chat_cat(254) | nm Chat
NM_OFFS = {4: 512, 5: 1024, 6: 1280}   # node-major Chat blocks (m x 64)
# wint split point per group size: tiles [gs:T] multiply on gpsimd (runs
# concurrently with the DVE, which keeps tiles [0:gs] plus all level-0 ops)
GP_SPLIT = {}  # gpsimd offload measured net-negative: DVE TTs lose rd1 bandwidth

EPS = 1e-6
SHARPNESS = 1.0
BIAS_SHIFT = -2.0

# dtypes (tunable): matmul path and tree path
MM_DT = mybir.dt.bfloat16    # halves x DMA and runs PE at full rate
TREE_DT = mybir.dt.float16   # 16-bit => DVE 2x mode on contiguous tensor ops
CST_DT = mybir.dt.float16    # tree constants (must match tree dtype for tt)


def _sigmoid(z):
    return 1.0 / (1.0 + np.exp(-z))


def _levels():
    """[(offset, m)] for m = 128, 64, ..., 1 into the weights/biases arrays."""
    out, off, m = [], 0, 128
    while m >= 1:
        out.append((off, m))
        off += m
        m //= 2
    return out


def _bitrev(n):
    """Bit-reversal permutation of 0..n-1 (involution)."""
    bits = n.bit_length() - 1
    out = np.zeros(n, np.int64)
    for j in range(n):
        r, x = 0, j
        for _ in range(bits):
            r = (r << 1) | (x & 1)
            x >>= 1
        out[j] = r
    return out


def prep_consts(weights, biases, w_out):
    """Host-folded per-node constants, A-folded, in bit-reversed order.

    Each node's A = lam*k coefficient is pushed down into its children's
    scales (sigma chain, anchored at sigma_root = RHO for fp16 range), so
    levels 0..6 need only  node = (l + r) + Chat*max(l, r)  with
    Chat = C/A.  The root level keeps explicit A'=A/RHO, C'=C/RHO.

    Level li stores its m output nodes at position q = bitrev(j); with leaves
    stored bit-reversed too, every level's children are the two contiguous
    halves of the previous buffer, elementwise aligned with the outputs.

    Returns (wint[256], Chat_cat[254], a7, c7) in float64 (orig math),
    br-permuted, ready to cast.
    """
    w = weights.astype(np.float64)
    b = biases.astype(np.float64)
    lv = _levels()
    A_lv, C_lv = [], []
    for li, (off, m) in enumerate(lv):
        lam = _sigmoid(b[off : off + m])
        if li + 1 < len(lv):
            noff, nm = lv[li + 1]
            k = np.empty(m, np.float64)
            k[0::2] = w[noff : noff + nm, 0]
            k[1::2] = w[noff : noff + nm, 1]
        else:
            k = np.full(m, float(w_out[0, 0]), np.float64)
        A_lv.append(lam * k)
        C_lv.append(k * (1.0 - 2.0 * lam))
    # sigma chain: sig[li][j] = scale of level-li node j's stored value.
    # Levels 0..6 use the A-folded 4-op form, so each level's A goes into its
    # children's sigma; the root (level 7) keeps its A explicit, so its
    # children carry only the RHO range-rescale.
    sig = [None] * 7
    sig[6] = np.full(2, RHO)
    for li in range(5, -1, -1):
        j = np.arange(128 >> li)
        sig[li] = sig[li + 1][j >> 1] * A_lv[li + 1][j >> 1]
    l_idx = np.arange(256)
    leaf_sig = sig[0][l_idx >> 1] * A_lv[0][l_idx >> 1]

    wint = np.empty(256, np.float64)
    wint[0::2] = w[0:128, 0]
    wint[1::2] = w[0:128, 1]
    wint = (wint * leaf_sig)[_bitrev(256)]

    Chat_parts = [
        (C_lv[li] / A_lv[li])[_bitrev(128 >> li)] for li in range(7)
    ]
    a7 = float(A_lv[7][0] / RHO)
    c7 = float(C_lv[7][0] / RHO)
    return wint, np.concatenate(Chat_parts), a7, c7


def host_emulate(x, W_leaf, weights, biases, w_out, b_out, dtype=np.float32):
    """Pure-numpy emulation of the exact kernel math/layout (for validation)."""
    wint, Chat_cat, a7, c7 = prep_consts(weights, biases, w_out)
    W_perm = W_leaf[_bitrev(256)]  # leaf l lands in column bitrev(l)
    leaf = _sigmoid(
        (x.astype(np.float32) @ W_perm.T.astype(np.float32)) + np.float32(BIAS_SHIFT)
    ).astype(dtype)
    cur = (leaf * wint.astype(dtype)).astype(dtype)
    off = 0
    for li in range(7):
        m = 128 >> li
        l_, r_ = cur[:, 0:m], cur[:, m : 2 * m]
        s = (l_ + r_).astype(dtype)
        mx = np.maximum(l_, r_)
        Ch = Chat_cat[off : off + m].astype(dtype)
        cur = (s + (mx * Ch).astype(dtype)).astype(dtype)
        off += m
    l_, r_ = cur[:, 0:1], cur[:, 1:2]
    s = (l_ + r_).astype(dtype)
    mx = np.maximum(l_, r_)
    cur = ((s * dtype(a7)).astype(dtype) + (mx * dtype(c7)).astype(dtype)).astype(dtype)
    return _sigmoid(cur.astype(np.float32) + np.float32(b_out[0]))


def build_nc(b_out_val, a7, c7):
    """Build the per-core Bass program (SPMD; same NEFF on all cores)."""
    nc = bacc.Bacc("TRN2", target_bir_lowering=False, debug=False)

    xt = nc.dram_tensor("xt", [128, 2, BS], MM_DT, kind="ExternalInput")
    wt = nc.dram_tensor("wt", [128, 2, 256], MM_DT, kind="ExternalInput")
    # cst rows all identical: [wint'(256) | Chat_cat(254) | pad]
    cst = nc.dram_tensor("cst", [128, CST_W], CST_DT, kind="ExternalInput")
    outp = nc.dram_tensor("out", [128, TILES], mybir.dt.float32, kind="ExternalOutput")

    CHAT_OFF = 256
    XCOLS = XSUB * 128

    with tile.TileContext(nc) as tc:
        with (
            tc.tile_pool(name="const", bufs=1) as constp,
            tc.tile_pool(name="xload", bufs=5) as xp,
            tc.tile_pool(name="leaf", bufs=3) as leafp,
            tc.tile_pool(name="work", bufs=1) as workp,
            tc.tile_pool(name="psum", bufs=3, space="PSUM") as psp,
            tc.tile_pool(name="warmpsum", bufs=1, space="PSUM") as wpsp,
        ):
            # x chunk DMAs go first on the sync queue (they gate everything);
            # wt/cst ride the gpsimd queue in parallel. Each DMA has a ~3.4us
            # descriptor-latency floor, so both planes of a chunk share one.
            # DMA completion latency is a fixed ~4us, so the two gating
            # transfers (first x chunk, wt) go out immediately on separate
            # queues; cst follows on gpsimd (only needed by the first wint).
            xab_first = xp.tile([128, 2, XSUB * 128], MM_DT, tag="xab")
            fcols = GROUP_SIZES[0] * 128
            nc.sync.dma_start(
                out=xab_first[:, :, 0:fcols], in_=xt.ap()[:, :, 0:fcols]
            )
            wt_sb = constp.tile([128, 2, 256], MM_DT)
            cst_sb = constp.tile([128, CST_W], CST_DT)
            nc.gpsimd.dma_start(out=wt_sb[:, :, :], in_=wt.ap())
            nc.sync.dma_start(out=cst_sb[:, :], in_=cst.ap())
            roots = constp.tile([128, TILES], TREE_DT)
            bias_shift = constp.tile([128, 1], mybir.dt.float32)
            nc.vector.memset(bias_shift[:, :], float(BIAS_SHIFT))
            bias_out = constp.tile([128, 1], mybir.dt.float32)
            nc.vector.memset(bias_out[:, :], float(b_out_val))
            # Warm the ACT sigmoid table at t=0 so the ~2.7us table load
            # overlaps the first x DMA + matmuls instead of serializing in
            # front of the first real activation.
            warm = constp.tile([128, 1], mybir.dt.float32)
            nc.scalar.activation(
                out=warm[:, :],
                in_=bias_out[:, :],
                func=mybir.ActivationFunctionType.Sigmoid,
                bias=bias_out[:, :],
                scale=1.0,
            )
            # Warm the PE HAM clock gate during the initial DMA wait: ~16
            # back-to-back matmuls on a zeroed SBUF tile give the >3.4us of
            # sustained PE activity that flips the clock from 1.2 to 2.4 GHz
            # before the first real matmul issues.
            dummy = constp.tile([128, 256], MM_DT)
            nc.vector.memset(dummy[:, :], 0.0)
            wps = wpsp.tile([128, 2, 256], mybir.dt.float32, tag="warmps")
            for _ in range(8):
                nc.tensor.matmul(
                    wps[:, 0, :], dummy[:, 0:128], dummy[:, :],
                    start=True, stop=True,
                )
                nc.tensor.matmul(
                    wps[:, 1, :], dummy[:, 0:128], dummy[:, :],
                    start=True, stop=True,
                )
            # L0 outputs for all tiles, bit-reversed node order (the deep
            # pass consumes halves of the node dim, elementwise aligned).
            l0out = constp.tile([128, TILES, 128], TREE_DT)

            def bconst(lo, n, T):
                """cst slice [128, n] broadcast to [128, T, n]."""
                return (
                    cst_sb[:, lo : lo + n]
                    .rearrange("p (o w) -> p o w", o=1)
                    .broadcast_to([128, T, n])
                )

            gstart = 0
            chunk_no = 0
            for T in GROUP_SIZES:
                # x-load chunks of <= XSUB tiles
                chunks, rem = [], T
                while rem > 0:
                    c = min(XSUB, rem)
                    chunks.append(c)
                    rem -= c
                leafg = leafp.tile([128, T, 256], TREE_DT, tag="leafg")
                done = 0
                for xsub in chunks:
                    xcols = xsub * 128
                    xoff = gstart + done * 128
                    if gstart == 0 and done == 0:
                        xab = xab_first
                    else:
                        xab = xp.tile([128, 2, XCOLS], MM_DT, tag="xab")
                        dmaq = nc.sync if (chunk_no % 2 == 0) else nc.gpsimd
                        dmaq.dma_start(
                            out=xab[:, :, 0:xcols],
                            in_=xt.ap()[:, :, xoff : xoff + xcols],
                        )
                        chunk_no += 1
                    asub = min(ACT_SUB, xsub)
                    assert xsub % asub == 0
                    for tp in range(xsub // asub):
                        ps = psp.tile([128, asub, 256], mybir.dt.float32, tag="ps")
                        for half in range(asub):
                            tl = asub * tp + half  # tile within sub-block
                            bsl = slice(tl * 128, (tl + 1) * 128)
                            nc.tensor.matmul(
                                ps[:, half, :],
                                xab[:, 0, bsl],
                                wt_sb[:, 0, :],
                                start=True,
                                stop=False,
                            )
                            nc.tensor.matmul(
                                ps[:, half, :],
                                xab[:, 1, bsl],
                                wt_sb[:, 1, :],
                                start=False,
                                stop=True,
                            )
                        t0 = done + asub * tp
                        nc.scalar.activation(
                            out=leafg[:, t0 : t0 + asub, :],
                            in_=ps[:, :, :],
                            func=mybir.ActivationFunctionType.Sigmoid,
                            bias=bias_shift[:, :],
                            scale=float(SHARPNESS),
                        )
                    done += xsub

                # Per-group DVE work: wint mult + level 0 only (5 ops).
                # pg = leaf * wint' (sigma-folded level-0 child weights),
                # then level-0 nodes go straight into the shared l0out slab.
                pg = workp.tile([128, T, 256], TREE_DT, tag="pg", bufs=2)
                scr = workp.tile([128, T, 384], TREE_DT, tag="scratch", bufs=1)
                gs = GP_SPLIT.get(T, T)   # tiles [gs:T] go to gpsimd
                if gs < T:
                    nc.gpsimd.tensor_tensor(
                        out=pg[:, gs:T, :],
                        in0=leafg[:, gs:T, :],
                        in1=bconst(0, 256, T - gs),
                        op=mybir.AluOpType.mult,
                    )
                nc.vector.tensor_tensor(
                    out=pg[:, 0:gs, :],
                    in0=leafg[:, 0:gs, :],
                    in1=bconst(0, 256, gs),
                    op=mybir.AluOpType.mult,
                )
                le = pg[:, :, 0:128]
                ro = pg[:, :, 128:256]
                s = scr[:, :, 0:128]
                mx = scr[:, :, 128:256]
                q2 = scr[:, :, 256:384]
                nc.vector.tensor_tensor(out=s, in0=le, in1=ro, op=mybir.AluOpType.add)
                nc.vector.tensor_tensor(out=mx, in0=le, in1=ro, op=mybir.AluOpType.max)
                nc.vector.tensor_tensor(
                    out=q2,
                    in0=mx,
                    in1=bconst(CHAT_OFF, 128, T),
                    op=mybir.AluOpType.mult,
                )
                gt = gstart // 128
                nc.vector.tensor_tensor(
                    out=l0out[:, gt : gt + T, :],
                    in0=s,
                    in1=q2,
                    op=mybir.AluOpType.add,
                )
                gstart += T * 128

            # Deep pass: levels 1..6 once over all tiles (large FD => the
            # 58-cycle per-op startup is paid 24x instead of 8x per group),
            # then the root level and the output sigmoid. Levels 1..3 run
            # tile-major; level 3's output is written node-major (a strided
            # 1x write, +0.5us) so levels 4..6 and the root see contiguous
            # step-1 runs of 64 instead of runs of <=8 (saves ~2.5us).
            scrd = workp.tile([128, TILES, 192], TREE_DT, tag="dscratch", bufs=1)
            off = 128
            for li in range(1, 7):
                m = 128 >> li
                le = l0out[:, :, 0:m]
                ro = l0out[:, :, m : 2 * m]
                s = scrd[:, :, 0:m]
                mx = scrd[:, :, 64 : 64 + m]
                q2 = scrd[:, :, 128 : 128 + m]
                nc.vector.tensor_tensor(out=s, in0=le, in1=ro, op=mybir.AluOpType.add)
                nc.vector.tensor_tensor(out=mx, in0=le, in1=ro, op=mybir.AluOpType.max)
                nc.vector.tensor_tensor(
                    out=q2,
                    in0=mx,
                    in1=bconst(CHAT_OFF + off, m, TILES),
                    op=mybir.AluOpType.mult,
                )
                nc.vector.tensor_tensor(
                    out=l0out[:, :, 0:m], in0=s, in1=q2, op=mybir.AluOpType.add
                )
                off += m

            # root level: explicit A' = A/RHO, C' = C/RHO immediates
            s = scrd[:, :, 0:1]
            mx = scrd[:, :, 64:65]
            q2 = scrd[:, :, 128:129]
            nc.vector.tensor_tensor(
                out=s,
                in0=l0out[:, :, 0:1],
                in1=l0out[:, :, 1:2],
                op=mybir.AluOpType.add,
            )
            nc.vector.tensor_tensor(
                out=mx,
                in0=l0out[:, :, 0:1],
                in1=l0out[:, :, 1:2],
                op=mybir.AluOpType.max,
            )
            nc.vector.tensor_scalar_mul(out=q2, in0=mx, scalar1=float(c7))
            rsl = roots[:, :].rearrange("p (t o) -> p t o", o=1)
            nc.vector.scalar_tensor_tensor(
                out=rsl,
                in0=s,
                scalar=float(a7),
                in1=q2,
                op0=mybir.AluOpType.mult,
                op1=mybir.AluOpType.add,
            )

            final = constp.tile([128, TILES], mybir.dt.float32)
            nc.scalar.activation(
                out=final[:, :],
                in_=roots[:, :],
                func=mybir.ActivationFunctionType.Sigmoid,
                bias=bias_out[:, :],
                scale=1.0,
            )
            nc.sync.dma_start(out=outp.ap(), in_=final[:, :])

    nc.compile()
    return nc


def make_in_maps(x, W_leaf, weights, biases, w_out):
    """Host-side sharding + layout prep. Returns per-core input dicts."""
    import ml_dtypes

    np_mm = ml_dtypes.bfloat16
    np_cst = np.float16
    wint, Chat_cat, a7, c7 = prep_consts(weights, biases, w_out)

    cst_row = np.zeros(CST_W, np_cst)
    cst_row[0:256] = wint.astype(np_cst)
    cst_row[256 : 256 + 254] = Chat_cat.astype(np_cst)
    # node-major Chat for deep levels 4..6: value per node, repeated over
    # the 64-tile inner dim so the kernel-side AP is step-1
    lv_off = {1: 128, 2: 192, 3: 224, 4: 240, 5: 248, 6: 252}
    for li, nmo in NM_OFFS.items():
        m = 128 >> li
        ch = Chat_cat[lv_off[li] : lv_off[li] + m]
        cst_row[nmo : nmo + m * TILES] = np.repeat(ch, TILES).astype(np_cst)
    cst = np.ascontiguousarray(np.broadcast_to(cst_row, (128, CST_W)))

    # leaf l lands in column bitrev(l); wt[p, c, l] = W_perm[l, c*128 + p]
    W_perm = W_leaf[_bitrev(256)]
    WT = np.ascontiguousarray(W_perm.T.astype(np_mm))  # [256, 256] (k, l)
    wt_host = np.ascontiguousarray(WT.reshape(2, 128, 256).transpose(1, 0, 2))

    xT = np.ascontiguousarray(x.T.astype(np_mm))  # [256, B]
    in_maps = []
    for c in range(N_CORES):
        sh = np.ascontiguousarray(
            xT[:, c * BS : (c + 1) * BS].reshape(2, 128, BS).transpose(1, 0, 2)
        )
        in_maps.append({"xt": sh, "wt": wt_host, "cst": cst})
    return in_maps, a7, c7


def gather_out(results):
    """Per-core [128, TILES] outputs -> full [B, 1]."""
    full = np.empty((B, 1), np.float32)
    for c in range(N_CORES):
        r = np.asarray(results[c]["out"])  # [128, TILES]
        full[c * BS : (c + 1) * BS, 0] = r.T.reshape(BS)
    return full


def kernel(x, W_leaf, weights, biases, w_out, b_out, _run_kwargs=None):
    x = np.asarray(x, dtype=np.float32)
    W_leaf = np.asarray(W_leaf, dtype=np.float32)
    weights = np.asarray(weights, dtype=np.float32)
    biases = np.asarray(biases, dtype=np.float32)
    w_out = np.asarray(w_out, dtype=np.float32)
    b_out = np.asarray(b_out, dtype=np.float32)
    in_maps, a7, c7 = make_in_maps(x, W_leaf, weights, biases, w_out)
    nc = build_nc(float(b_out[0]), a7, c7)
    kw = dict(_run_kwargs or {})
    res = run_bass_kernel_spmd(nc, in_maps, core_ids=list(range(N_CORES)), **kw)
    out = gather_out(res.results)
    if _run_kwargs is not None:
        kernel.last_results = res
    return out


# revision 18
# speedup vs baseline: 1.0096x; 1.0041x over previous
"""Trainium2 Bass kernel for nn_BinaryTreeLogicNet.

Computes, for x:[B,256], W_leaf:[256,256], weights:[255,2], biases:[255],
w_out:[1,1], b_out:[1]:

    leaf = sigmoid(x @ W_leaf.T - 2)                       # (B, 256)
    8-level pairwise tree reduce with generalized-gcd nodes # (B, 1)
    out  = sigmoid(root * w_out + b_out)

Key transformations (all host-side constant folding; math exact to ~1e-6):
  - every tree value is positive (sigmoid outputs x positive weights), so
    the |.| is a no-op and min/max are plain min/max;
  - lam*min + (1-lam)*max  =  A*(l+r) + C*max(l,r), A = lam*k, C = k(1-2lam),
    where the consumer weight k of the NEXT level folds in (w_out at root);
  - each level's A further folds into its children's stored scale (sigma
    chain anchored at RHO), so a level is just  node = (l+r) + Chat*max(l,r)
    with Chat = C/A: 4 DVE tensor_tensor ops per level, all fp16 2x mode;
  - levels are stored in bit-reversed node order, which makes every level's
    children the two contiguous halves of the previous buffer, elementwise
    aligned with the outputs: no strided access anywhere;
  - the reference's +EPS contributes k*EPS per node (~1e-6 absolute at the
    root, far below fp16 storage rounding) and is dropped.

Schedule: the kernel is DVE-throughput-bound (the tree is ~43us of
tensor_tensor streaming per core at the fp16 2x mode rate). Per x-group
only the wint multiply + level 0 run on the DVE (5 ops); levels 1..6 and
the root run once over all 64 tiles at the end with large free dims. Small
leading groups get the DVE started ~3us into the kernel.

Sharding: pure data parallel over the batch dim across 8 cores. x is
transposed (and cast to bf16) on the host so the matmul contraction dim
lands on SBUF partitions and all DMA is contiguous.
"""

import numpy as np

import concourse.bass as bass
import concourse.bacc as bacc
import concourse.mybir as mybir
import concourse.tile as tile
from concourse.bass_utils import run_bass_kernel_spmd

# ---- problem geometry (hardcoded per contract) ----
B, L = 65536, 256
N_CORES = 8
BS = B // N_CORES            # 8192 rows per core
TILES = BS // 128            # 64 tiles of 128 rows
# Tree-group sizes (in 128-row tiles). Per group only wint + level 0 run on
# the DVE (5 ops); levels 1..root run once over all tiles at the end (the
# "deep pass"), so groups can be small without paying per-op overhead 8x.
# Small leading groups let the DVE start ~3us in instead of ~18us.
GROUP_SIZES = [2, 2, 4, 8, 16, 32]
XSUB = 8                     # x tiles per DMA chunk (pipelining granularity)
ACT_SUB = 4                  # tiles per PSUM tile / activation (FD=1024)
RHO = 128.0                  # global pow2 rescale for the A-folded tree
CST_W = 1536                 # cst row: wint(256) | Chat_cat(254) | nm Chat
NM_OFFS = {4: 512, 5: 1024, 6: 1280}   # node-major Chat blocks (m x 64)
# wint split point per group size: tiles [gs:T] multiply on gpsimd (runs
# concurrently with the DVE, which keeps tiles [0:gs] plus all level-0 ops)
GP_SPLIT = {}  # gpsimd offload measured net-negative: DVE TTs lose rd1 bandwidth

EPS = 1e-6
SHARPNESS = 1.0
BIAS_SHIFT = -2.0

# dtypes (tunable): matmul path and tree path
MM_DT = mybir.dt.bfloat16    # halves x DMA and runs PE at full rate
TREE_DT = mybir.dt.float16   # 16-bit => DVE 2x mode on contiguous tensor ops
CST_DT = mybir.dt.float16    # tree constants (must match tree dtype for tt)


def _sigmoid(z):
    return 1.0 / (1.0 + np.exp(-z))


def _levels():
    """[(offset, m)] for m = 128, 64, ..., 1 into the weights/biases arrays."""
    out, off, m = [], 0, 128
    while m >= 1:
        out.append((off, m))
        off += m
        m //= 2
    return out


def _bitrev(n):
    """Bit-reversal permutation of 0..n-1 (involution)."""
    bits = n.bit_length() - 1
    out = np.zeros(n, np.int64)
    for j in range(n):
        r, x = 0, j
        for _ in range(bits):
            r = (r << 1) | (x & 1)
            x >>= 1
        out[j] = r
    return out


def prep_consts(weights, biases, w_out):
    """Host-folded per-node constants, A-folded, in bit-reversed order.

    Each node's A = lam*k coefficient is pushed down into its children's
    scales (sigma chain, anchored at sigma_root = RHO for fp16 range), so
    levels 0..6 need only  node = (l + r) + Chat*max(l, r)  with
    Chat = C/A.  The root level keeps explicit A'=A/RHO, C'=C/RHO.

    Level li stores its m output nodes at position q = bitrev(j); with leaves
    stored bit-reversed too, every level's children are the two contiguous
    halves of the previous buffer, elementwise aligned with the outputs.

    Returns (wint[256], Chat_cat[254], a7, c7) in float64 (orig math),
    br-permuted, ready to cast.
    """
    w = weights.astype(np.float64)
    b = biases.astype(np.float64)
    lv = _levels()
    A_lv, C_lv = [], []
    for li, (off, m) in enumerate(lv):
        lam = _sigmoid(b[off : off + m])
        if li + 1 < len(lv):
            noff, nm = lv[li + 1]
            k = np.empty(m, np.float64)
            k[0::2] = w[noff : noff + nm, 0]
            k[1::2] = w[noff : noff + nm, 1]
        else:
            k = np.full(m, float(w_out[0, 0]), np.float64)
        A_lv.append(lam * k)
        C_lv.append(k * (1.0 - 2.0 * lam))
    # sigma chain: sig[li][j] = scale of level-li node j's stored value.
    # Levels 0..6 use the A-folded 4-op form, so each level's A goes into its
    # children's sigma; the root (level 7) keeps its A explicit, so its
    # children carry only the RHO range-rescale.
    sig = [None] * 7
    sig[6] = np.full(2, RHO)
    for li in range(5, -1, -1):
        j = np.arange(128 >> li)
        sig[li] = sig[li + 1][j >> 1] * A_lv[li + 1][j >> 1]
    l_idx = np.arange(256)
    leaf_sig = sig[0][l_idx >> 1] * A_lv[0][l_idx >> 1]

    wint = np.empty(256, np.float64)
    wint[0::2] = w[0:128, 0]
    wint[1::2] = w[0:128, 1]
    wint = (wint * leaf_sig)[_bitrev(256)]

    Chat_parts = [
        (C_lv[li] / A_lv[li])[_bitrev(128 >> li)] for li in range(7)
    ]
    a7 = float(A_lv[7][0] / RHO)
    c7 = float(C_lv[7][0] / RHO)
    return wint, np.concatenate(Chat_parts), a7, c7


def host_emulate(x, W_leaf, weights, biases, w_out, b_out, dtype=np.float32):
    """Pure-numpy emulation of the exact kernel math/layout (for validation)."""
    wint, Chat_cat, a7, c7 = prep_consts(weights, biases, w_out)
    W_perm = W_leaf[_bitrev(256)]  # leaf l lands in column bitrev(l)
    leaf = _sigmoid(
        (x.astype(np.float32) @ W_perm.T.astype(np.float32)) + np.float32(BIAS_SHIFT)
    ).astype(dtype)
    cur = (leaf * wint.astype(dtype)).astype(dtype)
    off = 0
    for li in range(7):
        m = 128 >> li
        l_, r_ = cur[:, 0:m], cur[:, m : 2 * m]
        s = (l_ + r_).astype(dtype)
        mx = np.maximum(l_, r_)
        Ch = Chat_cat[off : off + m].astype(dtype)
        cur = (s + (mx * Ch).astype(dtype)).astype(dtype)
        off += m
    l_, r_ = cur[:, 0:1], cur[:, 1:2]
    s = (l_ + r_).astype(dtype)
    mx = np.maximum(l_, r_)
    cur = ((s * dtype(a7)).astype(dtype) + (mx * dtype(c7)).astype(dtype)).astype(dtype)
    return _sigmoid(cur.astype(np.float32) + np.float32(b_out[0]))


def build_nc(b_out_val, a7, c7):
    """Build the per-core Bass program (SPMD; same NEFF on all cores)."""
    nc = bacc.Bacc("TRN2", target_bir_lowering=False, debug=False)

    xt = nc.dram_tensor("xt", [128, 2, BS], MM_DT, kind="ExternalInput")
    wt = nc.dram_tensor("wt", [128, 2, 256], MM_DT, kind="ExternalInput")
    # cst rows all identical: [wint'(256) | Chat_cat(254) | pad]
    cst = nc.dram_tensor("cst", [128, CST_W], CST_DT, kind="ExternalInput")
    outp = nc.dram_tensor("out", [128, TILES], mybir.dt.float32, kind="ExternalOutput")

    CHAT_OFF = 256
    XCOLS = XSUB * 128

    with tile.TileContext(nc) as tc:
        with (
            tc.tile_pool(name="const", bufs=1) as constp,
            tc.tile_pool(name="xload", bufs=5) as xp,
            tc.tile_pool(name="leaf", bufs=2) as leafp,
            tc.tile_pool(name="work", bufs=1) as workp,
            tc.tile_pool(name="psum", bufs=3, space="PSUM") as psp,
            tc.tile_pool(name="warmpsum", bufs=1, space="PSUM") as wpsp,
        ):
            # x chunk DMAs go first on the sync queue (they gate everything);
            # wt/cst ride the gpsimd queue in parallel. Each DMA has a ~3.4us
            # descriptor-latency floor, so both planes of a chunk share one.
            # DMA completion latency is a fixed ~4us, so the two gating
            # transfers (first x chunk, wt) go out immediately on separate
            # queues; cst follows on gpsimd (only needed by the first wint).
            xab_first = xp.tile([128, 2, XSUB * 128], MM_DT, tag="xab")
            fcols = GROUP_SIZES[0] * 128
            nc.sync.dma_start(
                out=xab_first[0:64, :, 0:fcols], in_=xt.ap()[0:64, :, 0:fcols]
            )
            nc.gpsimd.dma_start(
                out=xab_first[64:128, :, 0:fcols], in_=xt.ap()[64:128, :, 0:fcols]
            )
            wt_sb = constp.tile([128, 2, 256], MM_DT)
            cst_sb = constp.tile([128, CST_W], CST_DT)
            nc.gpsimd.dma_start(out=wt_sb[:, :, :], in_=wt.ap())
            nc.gpsimd.dma_start(out=cst_sb[:, :], in_=cst.ap())
            roots = constp.tile([128, TILES], TREE_DT)
            bias_shift = constp.tile([128, 1], mybir.dt.float32)
            nc.vector.memset(bias_shift[:, :], float(BIAS_SHIFT))
            bias_out = constp.tile([128, 1], mybir.dt.float32)
            nc.vector.memset(bias_out[:, :], float(b_out_val))
            # Warm the ACT sigmoid table at t=0 so the ~2.7us table load
            # overlaps the first x DMA + matmuls instead of serializing in
            # front of the first real activation.
            warm = constp.tile([128, 1], mybir.dt.float32)
            nc.scalar.activation(
                out=warm[:, :],
                in_=bias_out[:, :],
                func=mybir.ActivationFunctionType.Sigmoid,
                bias=bias_out[:, :],
                scale=1.0,
            )
            # Warm the PE HAM clock gate during the initial DMA wait: ~16
            # back-to-back matmuls on a zeroed SBUF tile give the >3.4us of
            # sustained PE activity that flips the clock from 1.2 to 2.4 GHz
            # before the first real matmul issues.
            dummy = constp.tile([128, 256], MM_DT)
            nc.vector.memset(dummy[:, :], 0.0)
            wps = wpsp.tile([128, 2, 256], mybir.dt.float32, tag="warmps")
            for _ in range(8):
                nc.tensor.matmul(
                    wps[:, 0, :], dummy[:, 0:128], dummy[:, :],
                    start=True, stop=True,
                )
                nc.tensor.matmul(
                    wps[:, 1, :], dummy[:, 0:128], dummy[:, :],
                    start=True, stop=True,
                )
            # L0 outputs for all tiles, bit-reversed node order (the deep
            # pass consumes halves of the node dim, elementwise aligned).
            l0out = constp.tile([128, TILES, 128], TREE_DT)

            def bconst(lo, n, T):
                """cst slice [128, n] broadcast to [128, T, n]."""
                return (
                    cst_sb[:, lo : lo + n]
                    .rearrange("p (o w) -> p o w", o=1)
                    .broadcast_to([128, T, n])
                )

            gstart = 0
            chunk_no = 0
            for T in GROUP_SIZES:
                # x-load chunks of <= XSUB tiles
                chunks, rem = [], T
                while rem > 0:
                    c = min(XSUB, rem)
                    chunks.append(c)
                    rem -= c
                leafg = leafp.tile([128, T, 256], TREE_DT, tag="leafg")
                done = 0
                for xsub in chunks:
                    xcols = xsub * 128
                    xoff = gstart + done * 128
                    if gstart == 0 and done == 0:
                        xab = xab_first
                    else:
                        xab = xp.tile([128, 2, XCOLS], MM_DT, tag="xab")
                        nc.sync.dma_start(
                            out=xab[:, :, 0:xcols],
                            in_=xt.ap()[:, :, xoff : xoff + xcols],
                        )
                    asub = min(ACT_SUB, xsub)
                    assert xsub % asub == 0
                    for tp in range(xsub // asub):
                        ps = psp.tile([128, asub, 256], mybir.dt.float32, tag="ps")
                        for half in range(asub):
                            tl = asub * tp + half  # tile within sub-block
                            bsl = slice(tl * 128, (tl + 1) * 128)
                            nc.tensor.matmul(
                                ps[:, half, :],
                                xab[:, 0, bsl],
                                wt_sb[:, 0, :],
                                start=True,
                                stop=False,
                            )
                            nc.tensor.matmul(
                                ps[:, half, :],
                                xab[:, 1, bsl],
                                wt_sb[:, 1, :],
                                start=False,
                                stop=True,
                            )
                        t0 = done + asub * tp
                        nc.scalar.activation(
                            out=leafg[:, t0 : t0 + asub, :],
                            in_=ps[:, :, :],
                            func=mybir.ActivationFunctionType.Sigmoid,
                            bias=bias_shift[:, :],
                            scale=float(SHARPNESS),
                        )
                    done += xsub

                # Per-group DVE work: wint mult + level 0 only (5 ops).
                # pg = leaf * wint' (sigma-folded level-0 child weights),
                # then level-0 nodes go straight into the shared l0out slab.
                pg = workp.tile([128, T, 256], TREE_DT, tag="pg", bufs=2)
                scr = workp.tile([128, T, 384], TREE_DT, tag="scratch", bufs=1)
                gs = GP_SPLIT.get(T, T)   # tiles [gs:T] go to gpsimd
                if gs < T:
                    nc.gpsimd.tensor_tensor(
                        out=pg[:, gs:T, :],
                        in0=leafg[:, gs:T, :],
                        in1=bconst(0, 256, T - gs),
                        op=mybir.AluOpType.mult,
                    )
                nc.vector.tensor_tensor(
                    out=pg[:, 0:gs, :],
                    in0=leafg[:, 0:gs, :],
                    in1=bconst(0, 256, gs),
                    op=mybir.AluOpType.mult,
                )
                le = pg[:, :, 0:128]
                ro = pg[:, :, 128:256]
                s = scr[:, :, 0:128]
                mx = scr[:, :, 128:256]
                q2 = scr[:, :, 256:384]
                nc.vector.tensor_tensor(out=s, in0=le, in1=ro, op=mybir.AluOpType.add)
                nc.vector.tensor_tensor(out=mx, in0=le, in1=ro, op=mybir.AluOpType.max)
                nc.vector.tensor_tensor(
                    out=q2,
                    in0=mx,
                    in1=bconst(CHAT_OFF, 128, T),
                    op=mybir.AluOpType.mult,
                )
                gt = gstart // 128
                nc.vector.tensor_tensor(
                    out=l0out[:, gt : gt + T, :],
                    in0=s,
                    in1=q2,
                    op=mybir.AluOpType.add,
                )
                gstart += T * 128

            # Deep pass: levels 1..6 once over all tiles (large FD => the
            # 58-cycle per-op startup is paid 24x instead of 8x per group),
            # then the root level and the output sigmoid. Levels 1..3 run
            # tile-major; level 3's output is written node-major (a strided
            # 1x write, +0.5us) so levels 4..6 and the root see contiguous
            # step-1 runs of 64 instead of runs of <=8 (saves ~2.5us).
            scrd = workp.tile([128, TILES, 192], TREE_DT, tag="dscratch", bufs=1)
            off = 128
            for li in range(1, 7):
                m = 128 >> li
                le = l0out[:, :, 0:m]
                ro = l0out[:, :, m : 2 * m]
                s = scrd[:, :, 0:m]
                mx = scrd[:, :, 64 : 64 + m]
                q2 = scrd[:, :, 128 : 128 + m]
                nc.vector.tensor_tensor(out=s, in0=le, in1=ro, op=mybir.AluOpType.add)
                nc.vector.tensor_tensor(out=mx, in0=le, in1=ro, op=mybir.AluOpType.max)
                nc.vector.tensor_tensor(
                    out=q2,
                    in0=mx,
                    in1=bconst(CHAT_OFF + off, m, TILES),
                    op=mybir.AluOpType.mult,
                )
                nc.vector.tensor_tensor(
                    out=l0out[:, :, 0:m], in0=s, in1=q2, op=mybir.AluOpType.add
                )
                off += m

            # root level: explicit A' = A/RHO, C' = C/RHO immediates
            s = scrd[:, :, 0:1]
            mx = scrd[:, :, 64:65]
            q2 = scrd[:, :, 128:129]
            nc.vector.tensor_tensor(
                out=s,
                in0=l0out[:, :, 0:1],
                in1=l0out[:, :, 1:2],
                op=mybir.AluOpType.add,
            )
            nc.vector.tensor_tensor(
                out=mx,
                in0=l0out[:, :, 0:1],
                in1=l0out[:, :, 1:2],
                op=mybir.AluOpType.max,
            )
            nc.vector.tensor_scalar_mul(out=q2, in0=mx, scalar1=float(c7))
            rsl = roots[:, :].rearrange("p (t o) -> p t o", o=1)
            nc.vector.scalar_tensor_tensor(
                out=rsl,
                in0=s,
                scalar=float(a7),
                in1=q2,
                op0=mybir.AluOpType.mult,
                op1=mybir.AluOpType.add,
            )

            final = constp.tile([128, TILES], mybir.dt.float32)
            nc.scalar.activation(
                out=final[:, :],
                in_=roots[:, :],
                func=mybir.ActivationFunctionType.Sigmoid,
                bias=bias_out[:, :],
                scale=1.0,
            )
            nc.sync.dma_start(out=outp.ap(), in_=final[:, :])

    nc.compile()
    return nc


def make_in_maps(x, W_leaf, weights, biases, w_out):
    """Host-side sharding + layout prep. Returns per-core input dicts."""
    import ml_dtypes

    np_mm = ml_dtypes.bfloat16
    np_cst = np.float16
    wint, Chat_cat, a7, c7 = prep_consts(weights, biases, w_out)

    cst_row = np.zeros(CST_W, np_cst)
    cst_row[0:256] = wint.astype(np_cst)
    cst_row[256 : 256 + 254] = Chat_cat.astype(np_cst)
    # node-major Chat for deep levels 4..6: value per node, repeated over
    # the 64-tile inner dim so the kernel-side AP is step-1
    lv_off = {1: 128, 2: 192, 3: 224, 4: 240, 5: 248, 6: 252}
    for li, nmo in NM_OFFS.items():
        m = 128 >> li
        ch = Chat_cat[lv_off[li] : lv_off[li] + m]
        cst_row[nmo : nmo + m * TILES] = np.repeat(ch, TILES).astype(np_cst)
    cst = np.ascontiguousarray(np.broadcast_to(cst_row, (128, CST_W)))

    # leaf l lands in column bitrev(l); wt[p, c, l] = W_perm[l, c*128 + p]
    W_perm = W_leaf[_bitrev(256)]
    WT = np.ascontiguousarray(W_perm.T.astype(np_mm))  # [256, 256] (k, l)
    wt_host = np.ascontiguousarray(WT.reshape(2, 128, 256).transpose(1, 0, 2))

    xT = np.ascontiguousarray(x.T.astype(np_mm))  # [256, B]
    in_maps = []
    for c in range(N_CORES):
        sh = np.ascontiguousarray(
            xT[:, c * BS : (c + 1) * BS].reshape(2, 128, BS).transpose(1, 0, 2)
        )
        in_maps.append({"xt": sh, "wt": wt_host, "cst": cst})
    return in_maps, a7, c7


def gather_out(results):
    """Per-core [128, TILES] outputs -> full [B, 1]."""
    full = np.empty((B, 1), np.float32)
    for c in range(N_CORES):
        r = np.asarray(results[c]["out"])  # [128, TILES]
        full[c * BS : (c + 1) * BS, 0] = r.T.reshape(BS)
    return full


def kernel(x, W_leaf, weights, biases, w_out, b_out, _run_kwargs=None):
    x = np.asarray(x, dtype=np.float32)
    W_leaf = np.asarray(W_leaf, dtype=np.float32)
    weights = np.asarray(weights, dtype=np.float32)
    biases = np.asarray(biases, dtype=np.float32)
    w_out = np.asarray(w_out, dtype=np.float32)
    b_out = np.asarray(b_out, dtype=np.float32)
    in_maps, a7, c7 = make_in_maps(x, W_leaf, weights, biases, w_out)
    nc = build_nc(float(b_out[0]), a7, c7)
    kw = dict(_run_kwargs or {})
    res = run_bass_kernel_spmd(nc, in_maps, core_ids=list(range(N_CORES)), **kw)
    out = gather_out(res.results)
    if _run_kwargs is not None:
        kernel.last_results = res
    return out
